# revision 14
# baseline (speedup 1.0000x reference)
"""Trainium2 Bass kernel: negative squared-distance VQ codebook scores.

score[b,t,k] = -precision * ||x[b,t] - codebook[k]||^2
             = 2p * (x.c) - p * ||x||^2 - p * ||c||^2

Strategy (8 NeuronCores, data-parallel over B):
  - Each core gets 2 batches = 2048 (b,t) rows of x; codebook replicated.
  - GEMM in bf16: psum = x.c - 0.5*||c||^2 (rank-1 update, p-independent).
  - x processed in groups of 4 row-tiles: one load, one cast, one crossbar
    transpose per group (dma_start_transpose has ~1.3us fixed cost).
  - Codebook cast writes an h-major layout so one crossbar transpose
    suffices; ||c||^2 from squares of the transposed codebook + ones
    matmuls.
  - Epilogue: out_bf16 = psum * 2p + (-p*||x||^2), ACT/DVE halves.
  - Output stored bf16 (halves HBM traffic), cast to f32 on host.
"""

from contextlib import ExitStack

import numpy as np

import concourse.bass as bass
import concourse.tile as tile
from concourse import bacc, mybir
from concourse.bass_utils import run_bass_kernel_spmd

B, T, D, K = 16, 1024, 256, 1024
N_CORES = 8
BT = B * T // N_CORES  # rows of x per core (2048)
P = 128                # partition tile
NT = BT // P           # bt tiles per core (16)
GT = 4                 # tiles per x group
NG = NT // GT          # x groups (4)
KT = K // P            # codebook column tiles (8)
KH = K // 2            # epilogue half (512)

F32 = mybir.dt.float32
BF16 = mybir.dt.bfloat16
AF = mybir.ActivationFunctionType
OP = mybir.AluOpType


def _build_kernel(ctx: ExitStack, tc: tile.TileContext, x_in, cb_in, p_in, out):
    nc = tc.nc

    singles = ctx.enter_context(tc.tile_pool(name="singles", bufs=1))
    xn_pool = ctx.enter_context(tc.tile_pool(name="xn", bufs=3))
    xbf_pool = ctx.enter_context(tc.tile_pool(name="xbf", bufs=3))
    xt_pool = ctx.enter_context(tc.tile_pool(name="xt", bufs=3))
    dump_pool = ctx.enter_context(tc.tile_pool(name="dump", bufs=2))
    small_pool = ctx.enter_context(tc.tile_pool(name="small", bufs=8))
    out_pool = ctx.enter_context(tc.tile_pool(name="outp", bufs=3))
    ps_pool = ctx.enter_context(tc.tile_pool(name="ps", bufs=4, space="PSUM"))
    psc_pool = ctx.enter_context(tc.tile_pool(name="psc", bufs=1, space="PSUM"))
    pre_ps_pool = ctx.enter_context(
        tc.tile_pool(name="pre_ps", bufs=2, space="PSUM")
    )

    p_bc = singles.tile([P, 1], F32)
    nc.sync.dma_start(out=p_bc, in_=p_in.to_broadcast([P, 1]))
    # identity for PE transposes (bf16: 1 cycle/column)
    ident = singles.tile([P, P], BF16)
    nc.gpsimd.memset(ident, 0.0)
    nc.gpsimd.affine_select(
        out=ident, in_=ident, compare_op=OP.not_equal, fill=1.0, base=0,
        pattern=[[-1, P]], channel_multiplier=1,
    )

    # ---- x group loads on sync; codebook loads on scalar ----
    xn_tiles = {}

    def load_xg(g):
        t = xn_pool.tile([P, GT, D], F32, name=f"xn{g}", tag="xn")
        nc.sync.dma_start(
            out=t,
            in_=x_in[g * GT * P : (g + 1) * GT * P, :].rearrange(
                "(j p) d -> p j d", p=P
            ),
        )
        xn_tiles[g] = t

    load_xg(0)
    load_xg(1)

    cbn = singles.tile([P, 2, 4, D], F32)  # [p, half, j, d]; k-tile = 4*half+j
    for hl in range(2):
        nc.scalar.dma_start(
            out=cbn[:, hl, :, :],
            in_=cb_in[hl * 4 * P : (hl + 1) * 4 * P, :].rearrange(
                "(j p) d -> p j d", p=P
            ),
        )

    # ---- small constants ----
    two_p = singles.tile([P, 1], F32)
    nc.scalar.mul(two_p, p_bc, 2.0)  # first ACT op; fires table load
    neghalf = singles.tile([1, P], BF16)   # rank-1 row scale: -0.5
    nc.vector.memset(neghalf, -0.5)
    ones_col = singles.tile([P, 1], BF16)  # column-sum weights for ||c||^2
    nc.vector.memset(ones_col, 1.0)

    # ---- per-group x pipeline pieces ----
    xbf_tiles, xt_tiles, npxsq = {}, {}, {}

    def emit_cast(g):
        xbf2 = xbf_pool.tile([P, GT, D], BF16, name=f"xbf{g}", tag="xb")
        nc.vector.tensor_copy(xbf2, xn_tiles[g])
        xbf_tiles[g] = xbf2

    def emit_trans(g):
        xt2 = xt_pool.tile([P, 2 * GT, P], BF16, name=f"xt{g}", tag="xt")
        xbf2 = xbf_tiles[g]
        for jj in range(2 * GT):
            t_, h = jj // 2, jj % 2
            ps_t = pre_ps_pool.tile([P, P], BF16)
            nc.tensor.transpose(
                ps_t, xbf2[:, t_, h * P : (h + 1) * P], ident
            )
            nc.vector.tensor_copy(xt2[:, jj, :], ps_t)
        xt_tiles[g] = xt2

    def emit_xsq(i):
        g, t_ = i // GT, i % GT
        dmp = dump_pool.tile([P, D], BF16, name=f"dmp{i}", tag="dmp")
        xsq = small_pool.tile([P, 1], F32, name=f"xsq{i}", tag="xsq")
        nc.scalar.activation(
            out=dmp, in_=xn_tiles[g][:, t_, :], func=AF.Square, accum_out=xsq
        )
        npx = small_pool.tile([P, 1], F32, name=f"npx{i}", tag="npx")
        nc.gpsimd.tensor_scalar(
            out=npx, in0=xsq, scalar1=two_p, scalar2=-0.5,
            op0=OP.mult, op1=OP.mult,
        )
        npxsq[i] = npx

    # first x group: cast early (ahead of cb casts on DVE)
    emit_cast(0)

    # ---- codebook cast (h-major) + one crossbar transpose ----
    # cbbf[q, h, kt, pd] = cb[kt*128+q, h*128+pd]
    cbbf = singles.tile([P, 2, KT, P], BF16)
    for kt in range(KT):
        src = cbn[:, kt // 4, kt % 4, :].rearrange("q (h pd) -> q h pd", h=2)
        dst = cbbf[:, :, kt, :]
        if kt % 2 == 0:
            nc.scalar.copy(dst, src)
        else:
            nc.vector.tensor_copy(dst, src)
    # cbt[pd, h*8+kt, q] = cb[kt*128+q, h*128+pd]
    cbt = singles.tile([P, 2 * KT, P], BF16)
    for jj in range(2 * KT):
        h, kt = jj // KT, jj % KT
        ps_t = pre_ps_pool.tile([P, P], BF16)
        nc.tensor.transpose(ps_t, cbbf[:, h, kt, :], ident)
        nc.vector.tensor_copy(cbt[:, jj, :], ps_t)
    emit_trans(0)

    def cbt_h(h, kq):  # [128, 512] moving operand: d-half h, k cols kq*512..
        return cbt[:, h * KT + kq * 4 : h * KT + (kq + 1) * 4, :]

    # ---- ||c||^2 row from squares of cbt ----
    sqc = singles.tile([P, 2 * KT, P], BF16)
    csqrow = singles.tile([1, K], BF16)
    nc.scalar.activation(out=sqc[:, 0:KT, :], in_=cbt[:, 0:KT, :],
                         func=AF.Square)
    nc.vector.tensor_mul(sqc[:, KT:, :], cbt[:, KT:, :], cbt[:, KT:, :])
    for kq in range(2):
        ps_c = psc_pool.tile([1, KH], F32)
        for h in range(2):
            nc.tensor.matmul(
                ps_c, lhsT=ones_col,
                rhs=sqc[:, h * KT + kq * 4 : h * KT + (kq + 1) * 4, :],
                start=(h == 0), stop=(h == 1),
            )
        if kq == 0:
            nc.scalar.copy(csqrow[:, 0:KH], ps_c)
        else:
            nc.vector.tensor_copy(csqrow[:, KH:K], ps_c)

    for i in range(GT):
        emit_xsq(i)
    emit_cast(1)
    emit_trans(1)

    # ---- main loop: per tile, with per-tile interleaved prefetch ----
    out_tiles = {}

    def emit_mm_epi(i):
        g, t_ = i // GT, i % GT
        xt2 = xt_tiles[g]
        if t_ == 0:
            out_tiles[g] = out_pool.tile([P, GT, K], BF16, name=f"o{g}",
                                         tag="o")
        out2 = out_tiles[g]
        pss = [
            ps_pool.tile([P, KH], F32, name=f"ps{i}_{kq}", tag=f"ps{kq}",
                         bufs=2)
            for kq in range(2)
        ]
        for h in range(2):
            for kq in range(2):
                nc.tensor.matmul(
                    pss[kq], lhsT=xt2[:, 2 * t_ + h, :], rhs=cbt_h(h, kq),
                    start=(h == 0), stop=False,
                )
        for kq in range(2):
            nc.tensor.matmul(
                pss[kq], lhsT=neghalf,
                rhs=csqrow[:, kq * KH : (kq + 1) * KH],
                start=False, stop=True,
            )
        nc.scalar.activation(
            out=out2[:, t_, 0:KH], in_=pss[0], func=AF.Identity,
            bias=npxsq[i], scale=two_p,
        )
        nc.vector.tensor_scalar(
            out=out2[:, t_, KH:K], in0=pss[1],
            scalar1=two_p, scalar2=npxsq[i], op0=OP.mult, op1=OP.add,
        )
        if t_ % 2 == 1:
            pr = t_ // 2
            nc.sync.dma_start(
                out=out[(g * GT + 2 * pr) * P : (g * GT + 2 * pr + 2) * P,
                        :].rearrange("(j p) k -> p j k", p=P),
                in_=out2[:, 2 * pr : 2 * pr + 2, :],
            )

    for i in range(NT):
        # prefetch: keep 1.5-2 groups in flight
        pf = i + 2 * GT  # tile index 8 ahead
        if pf < NT:
            g = pf // GT
            if pf % GT == 0:
                load_xg(g)
            elif pf % GT == 1:
                emit_cast(g)
            elif pf % GT == 2:
                emit_trans(g)
        if i + GT < NT:
            emit_xsq(i + GT)
        emit_mm_epi(i)


def build_program():
    nc = bacc.Bacc(
        "TRN2", target_bir_lowering=False, debug=False, num_devices=N_CORES
    )
    x_in = nc.dram_tensor("x", [BT, D], F32, kind="ExternalInput").ap()
    cb_in = nc.dram_tensor("codebook", [K, D], F32, kind="ExternalInput").ap()
    p_in = nc.dram_tensor("precision", [1, 1], F32, kind="ExternalInput").ap()
    out = nc.dram_tensor("out", [BT, K], BF16, kind="ExternalOutput").ap()

    with tile.TileContext(nc) as tc:
        with ExitStack() as ctx:
            _build_kernel(ctx, tc, x_in, cb_in, p_in, out)
    nc.compile()
    return nc


_PROGRAM = None


def _get_program():
    global _PROGRAM
    if _PROGRAM is None:
        _PROGRAM = build_program()
    return _PROGRAM


_RESET_DONE = False


def _reset_axon_device():
    """Best-effort terminal-side NRT reset: a previously crashed run can
    leave the NeuronCores in NRT_EXEC_UNIT_UNRECOVERABLE state."""
    global _RESET_DONE
    if _RESET_DONE:
        return
    _RESET_DONE = True
    try:
        import ctypes

        import jax

        jax.devices()  # ensure the PJRT client is initialized
        lib = ctypes.CDLL("/opt/axon/libaxon_pjrt.so")
        lib.axon_reset.restype = ctypes.c_int64
        lib.axon_reset()
    except Exception:
        pass


def kernel(x, codebook, precision, _trace=False):
    x = np.ascontiguousarray(np.asarray(x, dtype=np.float32))
    codebook = np.ascontiguousarray(np.asarray(codebook, dtype=np.float32))
    precision = np.ascontiguousarray(np.asarray(precision, dtype=np.float32))
    assert x.shape == (B, T, D) and codebook.shape == (K, D)

    _reset_axon_device()
    nc = _get_program()
    rows_per_core = B // N_CORES  # 2 batches per core
    in_maps = [
        {
            "x": x[c * rows_per_core : (c + 1) * rows_per_core].reshape(BT, D),
            "codebook": codebook,
            "precision": precision.reshape(1, 1),
        }
        for c in range(N_CORES)
    ]
    res = run_bass_kernel_spmd(
        nc, in_maps, core_ids=list(range(N_CORES)), trace=_trace
    )
    out = np.concatenate(
        [
            np.asarray(r["out"]).astype(np.float32).reshape(rows_per_core, T, K)
            for r in res.results
        ],
        axis=0,
    )
    if _trace:
        kernel.last_exec_time_ns = res.exec_time_ns
        kernel.last_results = res
    return out


if __name__ == "__main__":
    xs = np.random.randn(B, T, D).astype(np.float32)
    cb = np.random.randn(K, D).astype(np.float32)
    pr = np.ones((1,), dtype=np.float32)
    o = kernel(xs, cb, pr)
    print(o.shape, o.dtype)


# revision 15
# speedup vs baseline: 1.0867x; 1.0867x over previous
"""Trainium2 Bass kernel: negative squared-distance VQ codebook scores.

score[b,t,k] = -precision * ||x[b,t] - codebook[k]||^2
             = 2p * (x.c) - p * ||x||^2 - p * ||c||^2

Strategy (8 NeuronCores, data-parallel over B):
  - Each core gets 2 batches = 2048 (b,t) rows of x; codebook replicated.
  - GEMM in bf16: psum = x.c - 0.5*||c||^2 (rank-1 update, p-independent).
  - x processed in groups of 4 row-tiles: one load, one cast, one crossbar
    transpose per group (dma_start_transpose has ~1.3us fixed cost).
  - Codebook cast writes an h-major layout so one crossbar transpose
    suffices; ||c||^2 from squares of the transposed codebook + ones
    matmuls.
  - Epilogue: out_bf16 = psum * 2p + (-p*||x||^2), ACT/DVE halves.
  - Output stored bf16 (halves HBM traffic), cast to f32 on host.
"""

from contextlib import ExitStack

import numpy as np

import concourse.bass as bass
import concourse.tile as tile
from concourse import bacc, mybir
from concourse.bass_utils import run_bass_kernel_spmd

B, T, D, K = 16, 1024, 256, 1024
N_CORES = 8
BT = B * T // N_CORES  # rows of x per core (2048)
P = 128                # partition tile
NT = BT // P           # bt tiles per core (16)
GT = 4                 # tiles per x group
NG = NT // GT          # x groups (4)
KT = K // P            # codebook column tiles (8)
KH = K // 2            # epilogue half (512)

F32 = mybir.dt.float32
BF16 = mybir.dt.bfloat16
AF = mybir.ActivationFunctionType
OP = mybir.AluOpType


def _build_kernel(ctx: ExitStack, tc: tile.TileContext, x_in, cb_in, p_in, out):
    nc = tc.nc

    singles = ctx.enter_context(tc.tile_pool(name="singles", bufs=1))
    xn_pool = ctx.enter_context(tc.tile_pool(name="xn", bufs=3))
    xbf_pool = ctx.enter_context(tc.tile_pool(name="xbf", bufs=3))
    xt_pool = ctx.enter_context(tc.tile_pool(name="xt", bufs=3))
    dump_pool = ctx.enter_context(tc.tile_pool(name="dump", bufs=2))
    small_pool = ctx.enter_context(tc.tile_pool(name="small", bufs=8))
    out_pool = ctx.enter_context(tc.tile_pool(name="outp", bufs=3))
    ps_pool = ctx.enter_context(tc.tile_pool(name="ps", bufs=4, space="PSUM"))
    psc_pool = ctx.enter_context(tc.tile_pool(name="psc", bufs=1, space="PSUM"))
    pre_ps_pool = ctx.enter_context(
        tc.tile_pool(name="pre_ps", bufs=2, space="PSUM")
    )

    p_bc = singles.tile([P, 1], F32)
    nc.sync.dma_start(out=p_bc, in_=p_in.to_broadcast([P, 1]))
    # identity for PE transposes (bf16: 1 cycle/column)
    ident = singles.tile([P, P], BF16)
    nc.gpsimd.memset(ident, 0.0)
    nc.gpsimd.affine_select(
        out=ident, in_=ident, compare_op=OP.not_equal, fill=1.0, base=0,
        pattern=[[-1, P]], channel_multiplier=1,
    )

    # ---- x group loads on sync; codebook loads on scalar ----
    xn_tiles = {}

    def load_xg(g):
        t = xn_pool.tile([P, GT, D], F32, name=f"xn{g}", tag="xn")
        nc.sync.dma_start(
            out=t,
            in_=x_in[g * GT * P : (g + 1) * GT * P, :].rearrange(
                "(j p) d -> p j d", p=P
            ),
        )
        xn_tiles[g] = t

    load_xg(0)
    load_xg(1)

    cbn = singles.tile([P, 2, 4, D], F32)  # [p, half, j, d]; k-tile = 4*half+j
    for hl in range(2):
        nc.scalar.dma_start(
            out=cbn[:, hl, :, :],
            in_=cb_in[hl * 4 * P : (hl + 1) * 4 * P, :].rearrange(
                "(j p) d -> p j d", p=P
            ),
        )

    # ---- small constants ----
    two_p = singles.tile([P, 1], F32)
    nc.scalar.mul(two_p, p_bc, 2.0)  # first ACT op; fires table load
    neghalf = singles.tile([1, P], BF16)   # rank-1 row scale: -0.5
    nc.vector.memset(neghalf, -0.5)
    ones_col = singles.tile([P, 1], BF16)  # column-sum weights for ||c||^2
    nc.vector.memset(ones_col, 1.0)

    # ---- per-group x pipeline pieces ----
    xbf_tiles, xt_tiles, npxsq = {}, {}, {}

    def emit_cast(g):
        xbf2 = xbf_pool.tile([P, GT, D], BF16, name=f"xbf{g}", tag="xb")
        nc.vector.tensor_copy(xbf2, xn_tiles[g])
        xbf_tiles[g] = xbf2

    def emit_trans(g):
        xt2 = xt_pool.tile([P, 2 * GT, P], BF16, name=f"xt{g}", tag="xt")
        xbf2 = xbf_tiles[g]
        for jj in range(2 * GT):
            t_, h = jj // 2, jj % 2
            ps_t = pre_ps_pool.tile([P, P], BF16)
            nc.tensor.transpose(
                ps_t, xbf2[:, t_, h * P : (h + 1) * P], ident
            )
            if jj % 2 == 0:
                nc.scalar.copy(xt2[:, jj, :], ps_t)
            else:
                nc.vector.tensor_copy(xt2[:, jj, :], ps_t)
        xt_tiles[g] = xt2

    def emit_xsq(i):
        g, t_ = i // GT, i % GT
        dmp = dump_pool.tile([P, D], BF16, name=f"dmp{i}", tag="dmp")
        xsq = small_pool.tile([P, 1], F32, name=f"xsq{i}", tag="xsq")
        nc.scalar.activation(
            out=dmp, in_=xn_tiles[g][:, t_, :], func=AF.Square, accum_out=xsq
        )
        npx = small_pool.tile([P, 1], F32, name=f"npx{i}", tag="npx")
        nc.gpsimd.tensor_scalar(
            out=npx, in0=xsq, scalar1=two_p, scalar2=-0.5,
            op0=OP.mult, op1=OP.mult,
        )
        npxsq[i] = npx

    # first x group: cast early (ahead of cb casts on DVE)
    emit_cast(0)

    # ---- codebook cast (h-major) + one crossbar transpose ----
    # cbbf[q, h, kt, pd] = cb[kt*128+q, h*128+pd]
    cbbf = singles.tile([P, 2, KT, P], BF16)
    for kt in range(KT):
        src = cbn[:, kt // 4, kt % 4, :].rearrange("q (h pd) -> q h pd", h=2)
        dst = cbbf[:, :, kt, :]
        if kt % 2 == 0:
            nc.scalar.copy(dst, src)
        else:
            nc.vector.tensor_copy(dst, src)
    # cbt[pd, h*8+kt, q] = cb[kt*128+q, h*128+pd]
    cbt = singles.tile([P, 2 * KT, P], BF16)
    for jj in range(2 * KT):
        h, kt = jj // KT, jj % KT
        ps_t = pre_ps_pool.tile([P, P], BF16)
        nc.tensor.transpose(ps_t, cbbf[:, h, kt, :], ident)
        if jj % 2 == 0:
            nc.scalar.copy(cbt[:, jj, :], ps_t)
        else:
            nc.vector.tensor_copy(cbt[:, jj, :], ps_t)
    emit_trans(0)

    def cbt_h(h, kq):  # [128, 512] moving operand: d-half h, k cols kq*512..
        return cbt[:, h * KT + kq * 4 : h * KT + (kq + 1) * 4, :]

    for i in range(GT):
        emit_xsq(i)
    emit_cast(1)
    emit_trans(1)
    # ---- ||c||^2 row from squares of cbt ----
    sqc = singles.tile([P, 2 * KT, P], BF16)
    csqrow = singles.tile([1, K], BF16)
    nc.scalar.activation(out=sqc[:, 0:KT, :], in_=cbt[:, 0:KT, :],
                         func=AF.Square)
    nc.vector.tensor_mul(sqc[:, KT:, :], cbt[:, KT:, :], cbt[:, KT:, :])
    for kq in range(2):
        ps_c = psc_pool.tile([1, KH], F32)
        for h in range(2):
            nc.tensor.matmul(
                ps_c, lhsT=ones_col,
                rhs=sqc[:, h * KT + kq * 4 : h * KT + (kq + 1) * 4, :],
                start=(h == 0), stop=(h == 1),
            )
        if kq == 0:
            nc.scalar.copy(csqrow[:, 0:KH], ps_c)
        else:
            nc.vector.tensor_copy(csqrow[:, KH:K], ps_c)


    # ---- main loop: per tile, with per-tile interleaved prefetch ----
    out_tiles = {}

    def emit_mm_epi(i):
        g, t_ = i // GT, i % GT
        xt2 = xt_tiles[g]
        if t_ == 0:
            out_tiles[g] = out_pool.tile([P, GT, K], BF16, name=f"o{g}",
                                         tag="o")
        out2 = out_tiles[g]
        pss = [
            ps_pool.tile([P, KH], F32, name=f"ps{i}_{kq}", tag=f"ps{kq}",
                         bufs=2)
            for kq in range(2)
        ]
        for h in range(2):
            for kq in range(2):
                nc.tensor.matmul(
                    pss[kq], lhsT=xt2[:, 2 * t_ + h, :], rhs=cbt_h(h, kq),
                    start=(h == 0), stop=False,
                )
        for kq in range(2):
            nc.tensor.matmul(
                pss[kq], lhsT=neghalf,
                rhs=csqrow[:, kq * KH : (kq + 1) * KH],
                start=False, stop=True,
            )
        nc.scalar.activation(
            out=out2[:, t_, 0:KH], in_=pss[0], func=AF.Identity,
            bias=npxsq[i], scale=two_p,
        )
        nc.vector.tensor_scalar(
            out=out2[:, t_, KH:K], in0=pss[1],
            scalar1=two_p, scalar2=npxsq[i], op0=OP.mult, op1=OP.add,
        )
        if t_ % 2 == 1:
            pr = t_ // 2
            nc.sync.dma_start(
                out=out[(g * GT + 2 * pr) * P : (g * GT + 2 * pr + 2) * P,
                        :].rearrange("(j p) k -> p j k", p=P),
                in_=out2[:, 2 * pr : 2 * pr + 2, :],
            )

    for i in range(NT):
        # prefetch: keep 1.5-2 groups in flight
        pf = i + 2 * GT  # tile index 8 ahead
        if pf < NT:
            g = pf // GT
            if pf % GT == 0:
                load_xg(g)
            elif pf % GT == 1:
                emit_cast(g)
            elif pf % GT == 2:
                emit_trans(g)
        if i + GT < NT:
            emit_xsq(i + GT)
        emit_mm_epi(i)


def build_program():
    nc = bacc.Bacc(
        "TRN2", target_bir_lowering=False, debug=False, num_devices=N_CORES
    )
    x_in = nc.dram_tensor("x", [BT, D], F32, kind="ExternalInput").ap()
    cb_in = nc.dram_tensor("codebook", [K, D], F32, kind="ExternalInput").ap()
    p_in = nc.dram_tensor("precision", [1, 1], F32, kind="ExternalInput").ap()
    out = nc.dram_tensor("out", [BT, K], BF16, kind="ExternalOutput").ap()

    with tile.TileContext(nc) as tc:
        with ExitStack() as ctx:
            _build_kernel(ctx, tc, x_in, cb_in, p_in, out)
    nc.compile()
    return nc


_PROGRAM = None


def _get_program():
    global _PROGRAM
    if _PROGRAM is None:
        _PROGRAM = build_program()
    return _PROGRAM


_RESET_DONE = False


def _reset_axon_device():
    """Best-effort terminal-side NRT reset: a previously crashed run can
    leave the NeuronCores in NRT_EXEC_UNIT_UNRECOVERABLE state."""
    global _RESET_DONE
    if _RESET_DONE:
        return
    _RESET_DONE = True
    try:
        import ctypes

        import jax

        jax.devices()  # ensure the PJRT client is initialized
        lib = ctypes.CDLL("/opt/axon/libaxon_pjrt.so")
        lib.axon_reset.restype = ctypes.c_int64
        lib.axon_reset()
    except Exception:
        pass


def kernel(x, codebook, precision, _trace=False):
    x = np.ascontiguousarray(np.asarray(x, dtype=np.float32))
    codebook = np.ascontiguousarray(np.asarray(codebook, dtype=np.float32))
    precision = np.ascontiguousarray(np.asarray(precision, dtype=np.float32))
    assert x.shape == (B, T, D) and codebook.shape == (K, D)

    _reset_axon_device()
    nc = _get_program()
    rows_per_core = B // N_CORES  # 2 batches per core
    in_maps = [
        {
            "x": x[c * rows_per_core : (c + 1) * rows_per_core].reshape(BT, D),
            "codebook": codebook,
            "precision": precision.reshape(1, 1),
        }
        for c in range(N_CORES)
    ]
    res = run_bass_kernel_spmd(
        nc, in_maps, core_ids=list(range(N_CORES)), trace=_trace
    )
    out = np.concatenate(
        [
            np.asarray(r["out"]).astype(np.float32).reshape(rows_per_core, T, K)
            for r in res.results
        ],
        axis=0,
    )
    if _trace:
        kernel.last_exec_time_ns = res.exec_time_ns
        kernel.last_results = res
    return out


if __name__ == "__main__":
    xs = np.random.randn(B, T, D).astype(np.float32)
    cb = np.random.randn(K, D).astype(np.float32)
    pr = np.ones((1,), dtype=np.float32)
    o = kernel(xs, cb, pr)
    print(o.shape, o.dtype)


# revision 16
# speedup vs baseline: 1.1140x; 1.0252x over previous
"""Trainium2 Bass kernel: negative squared-distance VQ codebook scores.

score[b,t,k] = -precision * ||x[b,t] - codebook[k]||^2
             = 2p * (x.c) - p * ||x||^2 - p * ||c||^2

Strategy (8 NeuronCores, data-parallel over B):
  - Each core gets 2 batches = 2048 (b,t) rows of x; codebook replicated.
  - GEMM in bf16: psum = x.c - 0.5*||c||^2 (rank-1 update, p-independent).
  - x processed in groups of 4 row-tiles: one load, one cast, one crossbar
    transpose per group (dma_start_transpose has ~1.3us fixed cost).
  - Codebook cast writes an h-major layout so one crossbar transpose
    suffices; ||c||^2 from squares of the transposed codebook + ones
    matmuls.
  - Epilogue: out_bf16 = psum * 2p + (-p*||x||^2), ACT/DVE halves.
  - Output stored bf16 (halves HBM traffic), cast to f32 on host.
"""

from contextlib import ExitStack

import numpy as np

import concourse.bass as bass
import concourse.tile as tile
from concourse import bacc, mybir
from concourse.bass_utils import run_bass_kernel_spmd

B, T, D, K = 16, 1024, 256, 1024
N_CORES = 8
BT = B * T // N_CORES  # rows of x per core (2048)
P = 128                # partition tile
NT = BT // P           # bt tiles per core (16)
GT = 4                 # tiles per x group
NG = NT // GT          # x groups (4)
KT = K // P            # codebook column tiles (8)
KH = K // 2            # epilogue half (512)

F32 = mybir.dt.float32
BF16 = mybir.dt.bfloat16
AF = mybir.ActivationFunctionType
OP = mybir.AluOpType


def _build_kernel(ctx: ExitStack, tc: tile.TileContext, x_in, cb_in, p_in, out):
    nc = tc.nc

    singles = ctx.enter_context(tc.tile_pool(name="singles", bufs=1))
    xn_pool = ctx.enter_context(tc.tile_pool(name="xn", bufs=3))
    xbf_pool = ctx.enter_context(tc.tile_pool(name="xbf", bufs=3))
    xt_pool = ctx.enter_context(tc.tile_pool(name="xt", bufs=3))
    dump_pool = ctx.enter_context(tc.tile_pool(name="dump", bufs=2))
    small_pool = ctx.enter_context(tc.tile_pool(name="small", bufs=8))
    out_pool = ctx.enter_context(tc.tile_pool(name="outp", bufs=3))
    ps_pool = ctx.enter_context(tc.tile_pool(name="ps", bufs=4, space="PSUM"))
    psc_pool = ctx.enter_context(tc.tile_pool(name="psc", bufs=1, space="PSUM"))
    pre_ps_pool = ctx.enter_context(
        tc.tile_pool(name="pre_ps", bufs=2, space="PSUM")
    )

    p_bc = singles.tile([P, 1], F32)
    nc.sync.dma_start(out=p_bc, in_=p_in.to_broadcast([P, 1]))
    # identity for PE transposes (bf16: 1 cycle/column)
    ident = singles.tile([P, P], BF16)
    nc.gpsimd.memset(ident, 0.0)
    nc.gpsimd.affine_select(
        out=ident, in_=ident, compare_op=OP.not_equal, fill=1.0, base=0,
        pattern=[[-1, P]], channel_multiplier=1,
    )

    # ---- x group loads on sync; codebook loads on scalar ----
    xn_tiles = {}

    def load_xg(g):
        t = xn_pool.tile([P, GT, D], F32, name=f"xn{g}", tag="xn")
        nc.sync.dma_start(
            out=t,
            in_=x_in[g * GT * P : (g + 1) * GT * P, :].rearrange(
                "(j p) d -> p j d", p=P
            ),
        )
        xn_tiles[g] = t

    load_xg(0)
    load_xg(1)

    cbn = singles.tile([P, 2, 4, D], F32)  # [p, half, j, d]; k-tile = 4*half+j
    for hl in range(2):
        nc.scalar.dma_start(
            out=cbn[:, hl, :, :],
            in_=cb_in[hl * 4 * P : (hl + 1) * 4 * P, :].rearrange(
                "(j p) d -> p j d", p=P
            ),
        )

    # ---- small constants ----
    two_p = singles.tile([P, 1], F32)
    nc.scalar.mul(two_p, p_bc, 2.0)  # first ACT op; fires table load
    neghalf = singles.tile([1, P], BF16)   # rank-1 row scale: -0.5
    nc.vector.memset(neghalf, -0.5)
    ones_col = singles.tile([P, 1], BF16)  # column-sum weights for ||c||^2
    nc.vector.memset(ones_col, 1.0)

    # ---- per-group x pipeline pieces ----
    xbf_tiles, xt_tiles, npxsq = {}, {}, {}

    def emit_cast(g):
        xbf2 = xbf_pool.tile([P, GT, D], BF16, name=f"xbf{g}", tag="xb")
        nc.vector.tensor_copy(xbf2, xn_tiles[g])
        xbf_tiles[g] = xbf2

    def emit_trans(g):
        xt2 = xt_pool.tile([P, 2 * GT, P], BF16, name=f"xt{g}", tag="xt")
        xbf2 = xbf_tiles[g]
        for jj in range(2 * GT):
            t_, h = jj // 2, jj % 2
            ps_t = pre_ps_pool.tile([P, P], BF16)
            nc.tensor.transpose(
                ps_t, xbf2[:, t_, h * P : (h + 1) * P], ident
            )
            nc.vector.tensor_copy(xt2[:, jj, :], ps_t)
        xt_tiles[g] = xt2

    def emit_xsq(i):
        g, t_ = i // GT, i % GT
        dmp = dump_pool.tile([P, D], BF16, name=f"dmp{i}", tag="dmp")
        xsq = small_pool.tile([P, 1], F32, name=f"xsq{i}", tag="xsq")
        nc.scalar.activation(
            out=dmp, in_=xn_tiles[g][:, t_, :], func=AF.Square, accum_out=xsq
        )
        npx = small_pool.tile([P, 1], F32, name=f"npx{i}", tag="npx")
        nc.gpsimd.tensor_scalar(
            out=npx, in0=xsq, scalar1=two_p, scalar2=-0.5,
            op0=OP.mult, op1=OP.mult,
        )
        npxsq[i] = npx

    # first x group: cast early (ahead of cb casts on DVE)
    emit_cast(0)

    # ---- codebook cast (h-major) + one crossbar transpose ----
    # cbbf[q, h, kt, pd] = cb[kt*128+q, h*128+pd]
    cbbf = singles.tile([P, 2, KT, P], BF16)
    for kt in range(KT):
        src = cbn[:, kt // 4, kt % 4, :].rearrange("q (h pd) -> q h pd", h=2)
        dst = cbbf[:, :, kt, :]
        if kt % 2 == 0:
            nc.scalar.copy(dst, src)
        else:
            nc.vector.tensor_copy(dst, src)
    # cbt[pd, h*8+kt, q] = cb[kt*128+q, h*128+pd]
    cbt = singles.tile([P, 2 * KT, P], BF16)
    for jj in range(2 * KT):
        h, kt = jj // KT, jj % KT
        ps_t = pre_ps_pool.tile([P, P], BF16)
        nc.tensor.transpose(ps_t, cbbf[:, h, kt, :], ident)
        if jj % 2 == 0:
            nc.scalar.copy(cbt[:, jj, :], ps_t)
        else:
            nc.vector.tensor_copy(cbt[:, jj, :], ps_t)
    emit_trans(0)

    def cbt_h(h, kq):  # [128, 512] moving operand: d-half h, k cols kq*512..
        return cbt[:, h * KT + kq * 4 : h * KT + (kq + 1) * 4, :]

    # ---- ||c||^2 row from squares of cbt ----
    sqc = singles.tile([P, 2 * KT, P], BF16)
    csqrow = singles.tile([1, K], BF16)
    nc.scalar.activation(out=sqc[:, 0:KT, :], in_=cbt[:, 0:KT, :],
                         func=AF.Square)
    nc.vector.tensor_mul(sqc[:, KT:, :], cbt[:, KT:, :], cbt[:, KT:, :])
    for kq in range(2):
        ps_c = psc_pool.tile([1, KH], F32)
        for h in range(2):
            nc.tensor.matmul(
                ps_c, lhsT=ones_col,
                rhs=sqc[:, h * KT + kq * 4 : h * KT + (kq + 1) * 4, :],
                start=(h == 0), stop=(h == 1),
            )
        if kq == 0:
            nc.scalar.copy(csqrow[:, 0:KH], ps_c)
        else:
            nc.vector.tensor_copy(csqrow[:, KH:K], ps_c)


    for i in range(GT):
        emit_xsq(i)
    emit_cast(1)
    emit_trans(1)
    # ---- main loop: per tile, with per-tile interleaved prefetch ----
    out_tiles = {}

    def emit_mm_epi(i):
        g, t_ = i // GT, i % GT
        xt2 = xt_tiles[g]
        if t_ == 0:
            out_tiles[g] = out_pool.tile([P, GT, K], BF16, name=f"o{g}",
                                         tag="o")
        out2 = out_tiles[g]
        pss = [
            ps_pool.tile([P, KH], F32, name=f"ps{i}_{kq}", tag=f"ps{kq}",
                         bufs=2)
            for kq in range(2)
        ]
        for h in range(2):
            for kq in range(2):
                nc.tensor.matmul(
                    pss[kq], lhsT=xt2[:, 2 * t_ + h, :], rhs=cbt_h(h, kq),
                    start=(h == 0), stop=False,
                )
        for kq in range(2):
            nc.tensor.matmul(
                pss[kq], lhsT=neghalf,
                rhs=csqrow[:, kq * KH : (kq + 1) * KH],
                start=False, stop=True,
            )
        nc.scalar.activation(
            out=out2[:, t_, 0:KH], in_=pss[0], func=AF.Identity,
            bias=npxsq[i], scale=two_p,
        )
        nc.vector.tensor_scalar(
            out=out2[:, t_, KH:K], in0=pss[1],
            scalar1=two_p, scalar2=npxsq[i], op0=OP.mult, op1=OP.add,
        )
        if t_ % 2 == 1:
            pr = t_ // 2
            nc.sync.dma_start(
                out=out[(g * GT + 2 * pr) * P : (g * GT + 2 * pr + 2) * P,
                        :].rearrange("(j p) k -> p j k", p=P),
                in_=out2[:, 2 * pr : 2 * pr + 2, :],
            )

    for i in range(NT):
        # prefetch: keep 1.5-2 groups in flight
        pf = i + 2 * GT  # tile index 8 ahead
        if pf < NT:
            g = pf // GT
            if pf % GT == 0:
                load_xg(g)
            elif pf % GT == 1:
                emit_cast(g)
            elif pf % GT == 2:
                emit_trans(g)
        if i + GT < NT:
            emit_xsq(i + GT)
        emit_mm_epi(i)


def build_program():
    nc = bacc.Bacc(
        "TRN2", target_bir_lowering=False, debug=False, num_devices=N_CORES
    )
    x_in = nc.dram_tensor("x", [BT, D], F32, kind="ExternalInput").ap()
    cb_in = nc.dram_tensor("codebook", [K, D], F32, kind="ExternalInput").ap()
    p_in = nc.dram_tensor("precision", [1, 1], F32, kind="ExternalInput").ap()
    out = nc.dram_tensor("out", [BT, K], BF16, kind="ExternalOutput").ap()

    with tile.TileContext(nc) as tc:
        with ExitStack() as ctx:
            _build_kernel(ctx, tc, x_in, cb_in, p_in, out)
    nc.compile()
    return nc


_PROGRAM = None


def _get_program():
    global _PROGRAM
    if _PROGRAM is None:
        _PROGRAM = build_program()
    return _PROGRAM


_RESET_DONE = False


def _reset_axon_device():
    """Best-effort terminal-side NRT reset: a previously crashed run can
    leave the NeuronCores in NRT_EXEC_UNIT_UNRECOVERABLE state."""
    global _RESET_DONE
    if _RESET_DONE:
        return
    _RESET_DONE = True
    try:
        import ctypes

        import jax

        jax.devices()  # ensure the PJRT client is initialized
        lib = ctypes.CDLL("/opt/axon/libaxon_pjrt.so")
        lib.axon_reset.restype = ctypes.c_int64
        lib.axon_reset()
    except Exception:
        pass


def kernel(x, codebook, precision, _trace=False):
    x = np.ascontiguousarray(np.asarray(x, dtype=np.float32))
    codebook = np.ascontiguousarray(np.asarray(codebook, dtype=np.float32))
    precision = np.ascontiguousarray(np.asarray(precision, dtype=np.float32))
    assert x.shape == (B, T, D) and codebook.shape == (K, D)

    _reset_axon_device()
    nc = _get_program()
    rows_per_core = B // N_CORES  # 2 batches per core
    in_maps = [
        {
            "x": x[c * rows_per_core : (c + 1) * rows_per_core].reshape(BT, D),
            "codebook": codebook,
            "precision": precision.reshape(1, 1),
        }
        for c in range(N_CORES)
    ]
    res = run_bass_kernel_spmd(
        nc, in_maps, core_ids=list(range(N_CORES)), trace=_trace
    )
    out = np.concatenate(
        [
            np.asarray(r["out"]).astype(np.float32).reshape(rows_per_core, T, K)
            for r in res.results
        ],
        axis=0,
    )
    if _trace:
        kernel.last_exec_time_ns = res.exec_time_ns
        kernel.last_results = res
    return out


if __name__ == "__main__":
    xs = np.random.randn(B, T, D).astype(np.float32)
    cb = np.random.randn(K, D).astype(np.float32)
    pr = np.ones((1,), dtype=np.float32)
    o = kernel(xs, cb, pr)
    print(o.shape, o.dtype)
